# revision 24
# baseline (speedup 1.0000x reference)
"""Distributed sparse-attention kernel for Trainium2 (8 NeuronCores, SPMD).

Computes, for x [8192, 256], adj [8192, 8192] (0/1 mask), Vw [256, 256], Vb [256]:
    value  = x @ Vw.T + Vb
    scores = (x @ x.T) / 16, masked where adj == 0
    p_attn = softmax(scores, axis=-1)
    Vs     = p_attn @ value
Returns (Vs [8192, 256] f32, p_attn [8192, 8192] f32).

Sharding: rows of x/adj across 8 cores (1024 rows each); x (keys) and value
replicated (small). Each core computes its [1024, 8192] probability block and
[1024, 256] output block independently — no collectives.
"""

import contextlib
import ctypes
import sys
import types

import numpy as np
import ml_dtypes

# ── Register the axon NTFF profile hook (image's antenv lacks axon_hooks) ──
def _make_hook(so_path):
    try:
        lib = ctypes.CDLL(so_path)
    except OSError:
        return None
    if not hasattr(lib, "axon_start_nrt_profile"):
        return None
    lib.axon_start_nrt_profile.argtypes = [ctypes.POINTER(ctypes.c_int64), ctypes.c_size_t]
    lib.axon_start_nrt_profile.restype = ctypes.c_int64
    lib.axon_stop_nrt_profile.argtypes = [ctypes.c_char_p]
    lib.axon_stop_nrt_profile.restype = ctypes.c_int64

    @contextlib.contextmanager
    def _hook(output_dir, device_ids):
        import jax
        jax.devices()
        if device_ids:
            ids = (ctypes.c_int64 * len(device_ids))(*device_ids)
            rc = lib.axon_start_nrt_profile(ids, len(device_ids))
        else:
            rc = lib.axon_start_nrt_profile(None, 0)
        if rc != 0:
            raise RuntimeError(f"axon_start_nrt_profile rc={rc}")
        try:
            yield
        finally:
            n = lib.axon_stop_nrt_profile(str(output_dir).encode())
            if n < 0:
                raise RuntimeError(f"axon_stop_nrt_profile rc={n}")
    return _hook


if "antenv.axon_hooks" not in sys.modules:
    _hooks_mod = types.ModuleType("antenv.axon_hooks")
    _HOOK = _make_hook("/opt/axon/libaxon_pjrt.so")
    _hooks_mod.get_axon_ntff_profile_hook = lambda: _HOOK
    _hooks_mod.set_axon_ntff_profile_hook = lambda h: None
    sys.modules["antenv.axon_hooks"] = _hooks_mod

from concourse import bacc, bass, masks, mybir, tile  # noqa: E402
from concourse import bass_utils  # noqa: E402
from concourse.bass_utils import run_bass_kernel_spmd  # noqa: E402

bass_utils.upload_artifacts = lambda tmpdir: f"local:{tmpdir}"

BF16 = mybir.dt.bfloat16
F32 = mybir.dt.float32

N = 8192          # tokens
DIN = 256         # features / d_model
M = 8             # cores
Q = N // M        # query rows per core (1024)
QB = 128          # query block (partition dim)
NQB = Q // QB     # 8 q-blocks per core
KC = 128          # k chunk (transpose/matmul2 granularity)
NKC = N // KC     # 64
KG = 512          # k group for scores matmul moving dim
NKG = N // KG     # 16
DAUG = DIN + 1    # value with ones column (row-sum trick)
SCALE = 1.0 / 16.0


def build_nc():
    nc = bacc.Bacc("TRN2", target_bir_lowering=False, debug=False, num_devices=M)

    # DRAM parameters (per core): same shapes on every core, different data.
    xt_d = nc.dram_tensor("xt", [128, 2, N], BF16, kind="ExternalInput").ap()
    xqt_d = nc.dram_tensor("xqt", [128, 2, Q], BF16, kind="ExternalInput").ap()
    vaug_d = nc.dram_tensor("vaug", [128, NKC, DAUG], BF16, kind="ExternalInput").ap()
    adjf_d = nc.dram_tensor("adjf", [Q, N], BF16, kind="ExternalInput").ap()
    p_d = nc.dram_tensor("p", [Q, N], F32, kind="ExternalOutput").ap()
    vs_d = nc.dram_tensor("vs", [Q, DIN], F32, kind="ExternalOutput").ap()

    with tile.TileContext(nc) as tc:
        with (
            tc.tile_pool(name="persist", bufs=1) as persist,
            tc.tile_pool(name="adjp", bufs=2) as adjp,
            tc.tile_pool(name="ep", bufs=2) as ep,
            tc.tile_pool(name="etp", bufs=2) as etp,
            tc.tile_pool(name="pfp", bufs=2) as pfp,
            tc.tile_pool(name="vsp", bufs=2) as vsp,
            tc.tile_pool(name="rp", bufs=2) as rp,
            tc.tile_pool(name="spsum", bufs=4, space="PSUM") as spsum,
            tc.tile_pool(name="tpsum", bufs=3, space="PSUM") as tpsum,
            tc.tile_pool(name="vpsum", bufs=1, space="PSUM") as vpsum,
        ):
            # ---- persistent loads ----
            # xt chunked per k-group so mm1 can start as soon as its slice lands.
            xqt_s = persist.tile([128, 2, Q], BF16)
            nc.sync.dma_start(xqt_s[:], xqt_d[:])
            xt_s = persist.tile([128, 2, N], BF16)
            for c in range(2):
                for h in range(2):
                    sl = slice(h * (N // 2), (h + 1) * (N // 2))
                    nc.sync.dma_start(xt_s[:, c, sl], xt_d[:, c, sl])
            vaug_s = persist.tile([128, NKC, DAUG], BF16)
            nc.sync.dma_start(vaug_s[:], vaug_d[:])
            ident = persist.tile([128, 128], BF16)
            masks.make_identity(nc, ident[:])

            # warm-up: junk matmuls during the input-DMA window keep the PE
            # HAM clock at 2.4 GHz for qb0's scores; a dummy activation pulls
            # the exp table load off the critical path.
            junk = persist.tile([128, 512], BF16)
            nc.gpsimd.memset(junk[:], 0.0)
            jout = persist.tile([128, 1], BF16)
            nc.scalar.activation(
                jout[:], junk[:, 0:1], mybir.ActivationFunctionType.Exp
            )
            wps = spsum.tile([QB, KG], F32, tag="sps")
            for w in range(20):
                nc.tensor.matmul(
                    wps[:], junk[:, 0:128], junk[:],
                    start=(w == 0), stop=(w == 19),
                )
            nc.scalar.activation(
                jout[:], wps[:, 0:1], mybir.ActivationFunctionType.Exp
            )

            def emit_mm1_pair(qb, e_s, adjf_s, kp):
                r0 = qb * QB
                ps_a = spsum.tile([QB, KG], F32, tag="sps")
                ps_b = spsum.tile([QB, KG], F32, tag="sps")
                pss = [ps_a, ps_b]
                for c in range(2):
                    for j, ps in enumerate(pss):
                        kg = 2 * kp + j
                        nc.tensor.matmul(
                            ps[:],
                            xqt_s[:, c, r0 : r0 + QB],
                            xt_s[:, c, kg * KG : (kg + 1) * KG],
                            start=(c == 0),
                            stop=(c == 1),
                        )
                for j, ps in enumerate(pss):
                    kg = 2 * kp + j
                    nc.scalar.activation(
                        e_s[:, kg * KG : (kg + 1) * KG],
                        ps[:],
                        mybir.ActivationFunctionType.Exp,
                        scale=SCALE,
                    )
                sl = slice(kp * 1024, (kp + 1) * 1024)
                nc.vector.tensor_mul(e_s[:, sl], e_s[:, sl], adjf_s[:, sl])

            def emit_transp_group(e_s, et_s, g):
                tp = tpsum.tile([128, 8, KC], BF16, tag="tp")
                for t in range(8):
                    c = 8 * g + t
                    nc.tensor.transpose(
                        tp[:, t, :], e_s[:, c * KC : (c + 1) * KC], ident[:]
                    )
                nc.vector.tensor_copy(et_s[:, 8 * g : 8 * g + 8, :], tp[:])

            def emit_mm2_outputs(qb, e_s, et_s):
                r0 = qb * QB
                vps = vpsum.tile([QB, DAUG], F32, tag="vps")
                for c in range(NKC):
                    nc.tensor.matmul(
                        vps[:],
                        et_s[:, c, :],
                        vaug_s[:, c, :],
                        start=(c == 0),
                        stop=(c == NKC - 1),
                    )
                recip = rp.tile([QB, 1], F32, tag="recip")
                nc.vector.reciprocal(recip[:], vps[:, DIN : DIN + 1])
                vs_s = vsp.tile([QB, DIN], F32, tag="vss")
                nc.scalar.activation(
                    vs_s[:],
                    vps[:, 0:DIN],
                    mybir.ActivationFunctionType.Copy,
                    scale=recip[:],
                )
                nc.sync.dma_start(vs_d[r0 : r0 + QB, :], vs_s[:])
                for h in range(4):
                    sl = slice(h * 2048, (h + 1) * 2048)
                    pf = pfp.tile([QB, 2048], F32, tag="pf")
                    nc.vector.tensor_scalar_mul(pf[:], e_s[:, sl], recip[:])
                    nc.sync.dma_start(p_d[r0 : r0 + QB, sl], pf[:])

            # PE stream per qb: mm1 pairs interleaved with the PREVIOUS
            # block's transposes (fills the ACT-paced exp window), then the
            # previous block's mm2 + outputs.
            prev = None
            for qb in range(NQB):
                adjf_s = adjp.tile([QB, N], BF16, tag="adjf")
                nc.sync.dma_start(adjf_s[:], adjf_d[qb * QB : (qb + 1) * QB, :])
                e_s = ep.tile([QB, N], BF16, tag="es")
                et_prev = None
                if prev is not None:
                    et_prev = etp.tile([128, NKC, KC], BF16, tag="ets")
                for kp in range(NKG // 2):
                    emit_mm1_pair(qb, e_s, adjf_s, kp)
                    if prev is not None:
                        emit_transp_group(prev[1], et_prev, kp)
                if prev is not None:
                    emit_mm2_outputs(prev[0], prev[1], et_prev)
                prev = (qb, e_s)

            # epilogue: last block's transposes + mm2
            et_last = etp.tile([128, NKC, KC], BF16, tag="ets")
            for g in range(NKC // 8):
                emit_transp_group(prev[1], et_last, g)
            emit_mm2_outputs(prev[0], prev[1], et_last)

    nc.compile()
    return nc


_NC_CACHE = None


def _get_nc():
    global _NC_CACHE
    if _NC_CACHE is None:
        _NC_CACHE = build_nc()
    return _NC_CACHE


def kernel(x, adj, Vw, Vb, _want_time=False):
    x = np.asarray(x, dtype=np.float32)
    adj_np = np.asarray(adj)
    Vw = np.asarray(Vw, dtype=np.float32)
    Vb = np.asarray(Vb, dtype=np.float32)

    # host-side prep (sharding + tiny value projection)
    value = x @ Vw.T + Vb                                    # [N, DIN] f32
    vaug = np.concatenate([value, np.ones((N, 1), np.float32)], axis=1)
    vaug_bf = np.ascontiguousarray(
        vaug.astype(ml_dtypes.bfloat16).reshape(NKC, KC, DAUG).transpose(1, 0, 2)
    )
    xt_bf = np.ascontiguousarray(
        x.T.astype(ml_dtypes.bfloat16).reshape(2, 128, N).transpose(1, 0, 2)
    )
    adj_bf = adj_np.astype(ml_dtypes.bfloat16)               # [N, N]

    in_maps = []
    for i in range(M):
        rows = slice(i * Q, (i + 1) * Q)
        xq = x[rows].T.astype(ml_dtypes.bfloat16)
        in_maps.append(
            {
                "xt": xt_bf,
                "xqt": np.ascontiguousarray(xq.reshape(2, 128, Q).transpose(1, 0, 2)),
                "vaug": vaug_bf,
                "adjf": np.ascontiguousarray(adj_bf[rows]),
            }
        )

    nc = _get_nc()
    res = run_bass_kernel_spmd(nc, in_maps, list(range(M)), trace=_want_time)

    p_attn = np.empty((N, N), np.float32)
    vs = np.empty((N, DIN), np.float32)
    for i in range(M):
        rows = slice(i * Q, (i + 1) * Q)
        p_attn[rows] = res.results[i]["p"]
        vs[rows] = res.results[i]["vs"]

    if _want_time:
        return (vs, p_attn), res.exec_time_ns
    return (vs, p_attn)


# revision 25
# speedup vs baseline: 1.0219x; 1.0219x over previous
"""Distributed sparse-attention kernel for Trainium2 (8 NeuronCores, SPMD).

Computes, for x [8192, 256], adj [8192, 8192] (0/1 mask), Vw [256, 256], Vb [256]:
    value  = x @ Vw.T + Vb
    scores = (x @ x.T) / 16, masked where adj == 0
    p_attn = softmax(scores, axis=-1)
    Vs     = p_attn @ value
Returns (Vs [8192, 256] f32, p_attn [8192, 8192] f32).

Sharding: rows of x/adj across 8 cores (1024 rows each); x (keys) and value
replicated (small). Each core computes its [1024, 8192] probability block and
[1024, 256] output block independently — no collectives.
"""

import contextlib
import ctypes
import sys
import types

import numpy as np
import ml_dtypes

# ── Register the axon NTFF profile hook (image's antenv lacks axon_hooks) ──
def _make_hook(so_path):
    try:
        lib = ctypes.CDLL(so_path)
    except OSError:
        return None
    if not hasattr(lib, "axon_start_nrt_profile"):
        return None
    lib.axon_start_nrt_profile.argtypes = [ctypes.POINTER(ctypes.c_int64), ctypes.c_size_t]
    lib.axon_start_nrt_profile.restype = ctypes.c_int64
    lib.axon_stop_nrt_profile.argtypes = [ctypes.c_char_p]
    lib.axon_stop_nrt_profile.restype = ctypes.c_int64

    @contextlib.contextmanager
    def _hook(output_dir, device_ids):
        import jax
        jax.devices()
        if device_ids:
            ids = (ctypes.c_int64 * len(device_ids))(*device_ids)
            rc = lib.axon_start_nrt_profile(ids, len(device_ids))
        else:
            rc = lib.axon_start_nrt_profile(None, 0)
        if rc != 0:
            raise RuntimeError(f"axon_start_nrt_profile rc={rc}")
        try:
            yield
        finally:
            n = lib.axon_stop_nrt_profile(str(output_dir).encode())
            if n < 0:
                raise RuntimeError(f"axon_stop_nrt_profile rc={n}")
    return _hook


if "antenv.axon_hooks" not in sys.modules:
    _hooks_mod = types.ModuleType("antenv.axon_hooks")
    _HOOK = _make_hook("/opt/axon/libaxon_pjrt.so")
    _hooks_mod.get_axon_ntff_profile_hook = lambda: _HOOK
    _hooks_mod.set_axon_ntff_profile_hook = lambda h: None
    sys.modules["antenv.axon_hooks"] = _hooks_mod

from concourse import bacc, bass, masks, mybir, tile  # noqa: E402
from concourse import bass_utils  # noqa: E402
from concourse.bass_utils import run_bass_kernel_spmd  # noqa: E402

bass_utils.upload_artifacts = lambda tmpdir: f"local:{tmpdir}"

BF16 = mybir.dt.bfloat16
F32 = mybir.dt.float32

N = 8192          # tokens
DIN = 256         # features / d_model
M = 8             # cores
Q = N // M        # query rows per core (1024)
QB = 128          # query block (partition dim)
NQB = Q // QB     # 8 q-blocks per core
KC = 128          # k chunk (transpose/matmul2 granularity)
NKC = N // KC     # 64
KG = 512          # k group for scores matmul moving dim
NKG = N // KG     # 16
DAUG = DIN + 1    # value with ones column (row-sum trick)
SCALE = 1.0 / 16.0


def build_nc():
    nc = bacc.Bacc("TRN2", target_bir_lowering=False, debug=False, num_devices=M)

    # DRAM parameters (per core): same shapes on every core, different data.
    xt_d = nc.dram_tensor("xt", [128, 2, N], BF16, kind="ExternalInput").ap()
    xqt_d = nc.dram_tensor("xqt", [128, 2, Q], BF16, kind="ExternalInput").ap()
    vaug_d = nc.dram_tensor("vaug", [128, NKC, DAUG], BF16, kind="ExternalInput").ap()
    adjf_d = nc.dram_tensor("adjf", [Q, N], BF16, kind="ExternalInput").ap()
    p_d = nc.dram_tensor("p", [Q, N], F32, kind="ExternalOutput").ap()
    vs_d = nc.dram_tensor("vs", [Q, DIN], F32, kind="ExternalOutput").ap()

    with tile.TileContext(nc) as tc:
        with (
            tc.tile_pool(name="persist", bufs=1) as persist,
            tc.tile_pool(name="adjp", bufs=2) as adjp,
            tc.tile_pool(name="ep", bufs=2) as ep,
            tc.tile_pool(name="etp", bufs=2) as etp,
            tc.tile_pool(name="pfp", bufs=2) as pfp,
            tc.tile_pool(name="vsp", bufs=2) as vsp,
            tc.tile_pool(name="rp", bufs=2) as rp,
            tc.tile_pool(name="spsum", bufs=5, space="PSUM") as spsum,
            tc.tile_pool(name="tpsum", bufs=2, space="PSUM") as tpsum,
            tc.tile_pool(name="vpsum", bufs=1, space="PSUM") as vpsum,
        ):
            # ---- persistent loads ----
            # xt chunked per k-group so mm1 can start as soon as its slice lands.
            xqt_s = persist.tile([128, 2, Q], BF16)
            nc.sync.dma_start(xqt_s[:], xqt_d[:])
            xt_s = persist.tile([128, 2, N], BF16)
            for c in range(2):
                for h in range(2):
                    sl = slice(h * (N // 2), (h + 1) * (N // 2))
                    nc.sync.dma_start(xt_s[:, c, sl], xt_d[:, c, sl])
            vaug_s = persist.tile([128, NKC, DAUG], BF16)
            nc.sync.dma_start(vaug_s[:], vaug_d[:])
            ident = persist.tile([128, 128], BF16)
            masks.make_identity(nc, ident[:])

            # warm-up: junk matmuls during the input-DMA window keep the PE
            # HAM clock at 2.4 GHz for qb0's scores; a dummy activation pulls
            # the exp table load off the critical path.
            junk = persist.tile([128, 512], BF16)
            nc.gpsimd.memset(junk[:], 0.0)
            jout = persist.tile([128, 1], BF16)
            nc.scalar.activation(
                jout[:], junk[:, 0:1], mybir.ActivationFunctionType.Exp
            )
            wps = spsum.tile([QB, KG], F32, tag="sps")
            for w in range(20):
                nc.tensor.matmul(
                    wps[:], junk[:, 0:128], junk[:],
                    start=(w == 0), stop=(w == 19),
                )
            nc.scalar.activation(
                jout[:], wps[:, 0:1], mybir.ActivationFunctionType.Exp
            )

            def emit_mm1_pair(qb, e_s, adjf_s, kp):
                r0 = qb * QB
                ps_a = spsum.tile([QB, KG], F32, tag="sps")
                ps_b = spsum.tile([QB, KG], F32, tag="sps")
                pss = [ps_a, ps_b]
                for c in range(2):
                    for j, ps in enumerate(pss):
                        kg = 2 * kp + j
                        nc.tensor.matmul(
                            ps[:],
                            xqt_s[:, c, r0 : r0 + QB],
                            xt_s[:, c, kg * KG : (kg + 1) * KG],
                            start=(c == 0),
                            stop=(c == 1),
                        )
                for j, ps in enumerate(pss):
                    kg = 2 * kp + j
                    nc.scalar.activation(
                        e_s[:, kg * KG : (kg + 1) * KG],
                        ps[:],
                        mybir.ActivationFunctionType.Exp,
                        scale=SCALE,
                    )
                sl = slice(kp * 1024, (kp + 1) * 1024)
                nc.vector.tensor_mul(e_s[:, sl], e_s[:, sl], adjf_s[:, sl])

            def emit_transp_group(e_s, et_s, g):
                tp = tpsum.tile([128, 8, KC], BF16, tag="tp")
                for t in range(8):
                    c = 8 * g + t
                    nc.tensor.transpose(
                        tp[:, t, :], e_s[:, c * KC : (c + 1) * KC], ident[:]
                    )
                nc.vector.tensor_copy(et_s[:, 8 * g : 8 * g + 8, :], tp[:])

            def emit_mm2_outputs(qb, e_s, et_s):
                r0 = qb * QB
                vps = vpsum.tile([QB, DAUG], F32, tag="vps")
                for c in range(NKC):
                    nc.tensor.matmul(
                        vps[:],
                        et_s[:, c, :],
                        vaug_s[:, c, :],
                        start=(c == 0),
                        stop=(c == NKC - 1),
                    )
                recip = rp.tile([QB, 1], F32, tag="recip")
                nc.vector.reciprocal(recip[:], vps[:, DIN : DIN + 1])
                vs_s = vsp.tile([QB, DIN], F32, tag="vss")
                nc.scalar.activation(
                    vs_s[:],
                    vps[:, 0:DIN],
                    mybir.ActivationFunctionType.Copy,
                    scale=recip[:],
                )
                nc.sync.dma_start(vs_d[r0 : r0 + QB, :], vs_s[:])
                for h in range(4):
                    sl = slice(h * 2048, (h + 1) * 2048)
                    pf = pfp.tile([QB, 2048], F32, tag="pf")
                    nc.vector.tensor_scalar_mul(pf[:], e_s[:, sl], recip[:])
                    nc.sync.dma_start(p_d[r0 : r0 + QB, sl], pf[:])

            # PE stream per qb: mm1 pairs interleaved with the PREVIOUS
            # block's transposes (fills the ACT-paced exp window), then the
            # previous block's mm2 + outputs.
            prev = None
            for qb in range(NQB):
                adjf_s = adjp.tile([QB, N], BF16, tag="adjf")
                nc.sync.dma_start(adjf_s[:], adjf_d[qb * QB : (qb + 1) * QB, :])
                e_s = ep.tile([QB, N], BF16, tag="es")
                et_prev = None
                if prev is not None:
                    et_prev = etp.tile([128, NKC, KC], BF16, tag="ets")
                for kp in range(NKG // 2):
                    emit_mm1_pair(qb, e_s, adjf_s, kp)
                    if prev is not None:
                        emit_transp_group(prev[1], et_prev, kp)
                if prev is not None:
                    emit_mm2_outputs(prev[0], prev[1], et_prev)
                prev = (qb, e_s)

            # epilogue: last block's transposes + mm2
            et_last = etp.tile([128, NKC, KC], BF16, tag="ets")
            for g in range(NKC // 8):
                emit_transp_group(prev[1], et_last, g)
            emit_mm2_outputs(prev[0], prev[1], et_last)

    nc.compile()
    return nc


_NC_CACHE = None


def _get_nc():
    global _NC_CACHE
    if _NC_CACHE is None:
        _NC_CACHE = build_nc()
    return _NC_CACHE


def kernel(x, adj, Vw, Vb, _want_time=False):
    x = np.asarray(x, dtype=np.float32)
    adj_np = np.asarray(adj)
    Vw = np.asarray(Vw, dtype=np.float32)
    Vb = np.asarray(Vb, dtype=np.float32)

    # host-side prep (sharding + tiny value projection)
    value = x @ Vw.T + Vb                                    # [N, DIN] f32
    vaug = np.concatenate([value, np.ones((N, 1), np.float32)], axis=1)
    vaug_bf = np.ascontiguousarray(
        vaug.astype(ml_dtypes.bfloat16).reshape(NKC, KC, DAUG).transpose(1, 0, 2)
    )
    xt_bf = np.ascontiguousarray(
        x.T.astype(ml_dtypes.bfloat16).reshape(2, 128, N).transpose(1, 0, 2)
    )
    adj_bf = adj_np.astype(ml_dtypes.bfloat16)               # [N, N]

    in_maps = []
    for i in range(M):
        rows = slice(i * Q, (i + 1) * Q)
        xq = x[rows].T.astype(ml_dtypes.bfloat16)
        in_maps.append(
            {
                "xt": xt_bf,
                "xqt": np.ascontiguousarray(xq.reshape(2, 128, Q).transpose(1, 0, 2)),
                "vaug": vaug_bf,
                "adjf": np.ascontiguousarray(adj_bf[rows]),
            }
        )

    nc = _get_nc()
    res = run_bass_kernel_spmd(nc, in_maps, list(range(M)), trace=_want_time)

    p_attn = np.empty((N, N), np.float32)
    vs = np.empty((N, DIN), np.float32)
    for i in range(M):
        rows = slice(i * Q, (i + 1) * Q)
        p_attn[rows] = res.results[i]["p"]
        vs[rows] = res.results[i]["vs"]

    if _want_time:
        return (vs, p_attn), res.exec_time_ns
    return (vs, p_attn)


# revision 26
# speedup vs baseline: 1.0315x; 1.0094x over previous
"""Distributed sparse-attention kernel for Trainium2 (8 NeuronCores, SPMD).

Computes, for x [8192, 256], adj [8192, 8192] (0/1 mask), Vw [256, 256], Vb [256]:
    value  = x @ Vw.T + Vb
    scores = (x @ x.T) / 16, masked where adj == 0
    p_attn = softmax(scores, axis=-1)
    Vs     = p_attn @ value
Returns (Vs [8192, 256] f32, p_attn [8192, 8192] f32).

Sharding: rows of x/adj across 8 cores (1024 rows each); x (keys) and value
replicated (small). Each core computes its [1024, 8192] probability block and
[1024, 256] output block independently — no collectives.
"""

import contextlib
import ctypes
import sys
import types

import numpy as np
import ml_dtypes

# ── Register the axon NTFF profile hook (image's antenv lacks axon_hooks) ──
def _make_hook(so_path):
    try:
        lib = ctypes.CDLL(so_path)
    except OSError:
        return None
    if not hasattr(lib, "axon_start_nrt_profile"):
        return None
    lib.axon_start_nrt_profile.argtypes = [ctypes.POINTER(ctypes.c_int64), ctypes.c_size_t]
    lib.axon_start_nrt_profile.restype = ctypes.c_int64
    lib.axon_stop_nrt_profile.argtypes = [ctypes.c_char_p]
    lib.axon_stop_nrt_profile.restype = ctypes.c_int64

    @contextlib.contextmanager
    def _hook(output_dir, device_ids):
        import jax
        jax.devices()
        if device_ids:
            ids = (ctypes.c_int64 * len(device_ids))(*device_ids)
            rc = lib.axon_start_nrt_profile(ids, len(device_ids))
        else:
            rc = lib.axon_start_nrt_profile(None, 0)
        if rc != 0:
            raise RuntimeError(f"axon_start_nrt_profile rc={rc}")
        try:
            yield
        finally:
            n = lib.axon_stop_nrt_profile(str(output_dir).encode())
            if n < 0:
                raise RuntimeError(f"axon_stop_nrt_profile rc={n}")
    return _hook


if "antenv.axon_hooks" not in sys.modules:
    _hooks_mod = types.ModuleType("antenv.axon_hooks")
    _HOOK = _make_hook("/opt/axon/libaxon_pjrt.so")
    _hooks_mod.get_axon_ntff_profile_hook = lambda: _HOOK
    _hooks_mod.set_axon_ntff_profile_hook = lambda h: None
    sys.modules["antenv.axon_hooks"] = _hooks_mod

from concourse import bacc, bass, masks, mybir, tile  # noqa: E402
from concourse import bass_utils  # noqa: E402
from concourse.bass_utils import run_bass_kernel_spmd  # noqa: E402

bass_utils.upload_artifacts = lambda tmpdir: f"local:{tmpdir}"

BF16 = mybir.dt.bfloat16
F32 = mybir.dt.float32

N = 8192          # tokens
DIN = 256         # features / d_model
M = 8             # cores
Q = N // M        # query rows per core (1024)
QB = 128          # query block (partition dim)
NQB = Q // QB     # 8 q-blocks per core
KC = 128          # k chunk (transpose/matmul2 granularity)
NKC = N // KC     # 64
KG = 512          # k group for scores matmul moving dim
NKG = N // KG     # 16
DAUG = DIN + 1    # value with ones column (row-sum trick)
SCALE = 1.0 / 16.0


def build_nc():
    nc = bacc.Bacc("TRN2", target_bir_lowering=False, debug=False, num_devices=M)

    # DRAM parameters (per core): same shapes on every core, different data.
    xt_d = nc.dram_tensor("xt", [128, 2, N], BF16, kind="ExternalInput").ap()
    xqt_d = nc.dram_tensor("xqt", [128, 2, Q], BF16, kind="ExternalInput").ap()
    vaug_d = nc.dram_tensor("vaug", [128, NKC, DAUG], BF16, kind="ExternalInput").ap()
    adjf_d = nc.dram_tensor("adjf", [Q, N], BF16, kind="ExternalInput").ap()
    p_d = nc.dram_tensor("p", [Q, N], F32, kind="ExternalOutput").ap()
    vs_d = nc.dram_tensor("vs", [Q, DIN], F32, kind="ExternalOutput").ap()

    with tile.TileContext(nc) as tc:
        with (
            tc.tile_pool(name="persist", bufs=1) as persist,
            tc.tile_pool(name="adjp", bufs=2) as adjp,
            tc.tile_pool(name="ep", bufs=2) as ep,
            tc.tile_pool(name="etp", bufs=2) as etp,
            tc.tile_pool(name="pfp", bufs=2) as pfp,
            tc.tile_pool(name="vsp", bufs=2) as vsp,
            tc.tile_pool(name="rp", bufs=2) as rp,
            tc.tile_pool(name="spsum", bufs=5, space="PSUM") as spsum,
            tc.tile_pool(name="tpsum", bufs=2, space="PSUM") as tpsum,
            tc.tile_pool(name="vpsum", bufs=1, space="PSUM") as vpsum,
        ):
            # ---- persistent loads ----
            # xt chunked per k-group so mm1 can start as soon as its slice lands.
            xqt_s = persist.tile([128, 2, Q], BF16)
            nc.sync.dma_start(xqt_s[:], xqt_d[:])
            xt_s = persist.tile([128, 2, N], BF16)
            for c in range(2):
                for h in range(2):
                    sl = slice(h * (N // 2), (h + 1) * (N // 2))
                    nc.sync.dma_start(xt_s[:, c, sl], xt_d[:, c, sl])
            vaug_s = persist.tile([128, NKC, DAUG], BF16)
            nc.sync.dma_start(vaug_s[:], vaug_d[:])
            ident = persist.tile([128, 128], BF16)
            masks.make_identity(nc, ident[:])

            # warm-up: junk matmuls during the input-DMA window keep the PE
            # HAM clock at 2.4 GHz for qb0's scores; a dummy activation pulls
            # the exp table load off the critical path.
            junk = persist.tile([128, 512], BF16)
            nc.gpsimd.memset(junk[:], 0.0)
            jout = persist.tile([128, 1], BF16)
            nc.scalar.activation(
                jout[:], junk[:, 0:1], mybir.ActivationFunctionType.Exp
            )
            wps = spsum.tile([QB, KG], F32, tag="sps")
            for w in range(34):
                nc.tensor.matmul(
                    wps[:], junk[:, 0:128], junk[:],
                    start=(w == 0), stop=(w == 33),
                )
            nc.scalar.activation(
                jout[:], wps[:, 0:1], mybir.ActivationFunctionType.Exp
            )

            def emit_mm1_pair(qb, e_s, adjf_s, kp):
                r0 = qb * QB
                ps_a = spsum.tile([QB, KG], F32, tag="sps")
                ps_b = spsum.tile([QB, KG], F32, tag="sps")
                pss = [ps_a, ps_b]
                for c in range(2):
                    for j, ps in enumerate(pss):
                        kg = 2 * kp + j
                        nc.tensor.matmul(
                            ps[:],
                            xqt_s[:, c, r0 : r0 + QB],
                            xt_s[:, c, kg * KG : (kg + 1) * KG],
                            start=(c == 0),
                            stop=(c == 1),
                        )
                for j, ps in enumerate(pss):
                    kg = 2 * kp + j
                    nc.scalar.activation(
                        e_s[:, kg * KG : (kg + 1) * KG],
                        ps[:],
                        mybir.ActivationFunctionType.Exp,
                        scale=SCALE,
                    )
                sl = slice(kp * 1024, (kp + 1) * 1024)
                nc.vector.tensor_mul(e_s[:, sl], e_s[:, sl], adjf_s[:, sl])

            def emit_transp_group(e_s, et_s, g):
                tp = tpsum.tile([128, 8, KC], BF16, tag="tp")
                for t in range(8):
                    c = 8 * g + t
                    nc.tensor.transpose(
                        tp[:, t, :], e_s[:, c * KC : (c + 1) * KC], ident[:]
                    )
                nc.vector.tensor_copy(et_s[:, 8 * g : 8 * g + 8, :], tp[:])

            def emit_mm2_outputs(qb, e_s, et_s):
                r0 = qb * QB
                vps = vpsum.tile([QB, DAUG], F32, tag="vps")
                for c in range(NKC):
                    nc.tensor.matmul(
                        vps[:],
                        et_s[:, c, :],
                        vaug_s[:, c, :],
                        start=(c == 0),
                        stop=(c == NKC - 1),
                    )
                recip = rp.tile([QB, 1], F32, tag="recip")
                nc.vector.reciprocal(recip[:], vps[:, DIN : DIN + 1])
                vs_s = vsp.tile([QB, DIN], F32, tag="vss")
                nc.scalar.activation(
                    vs_s[:],
                    vps[:, 0:DIN],
                    mybir.ActivationFunctionType.Copy,
                    scale=recip[:],
                )
                nc.sync.dma_start(vs_d[r0 : r0 + QB, :], vs_s[:])
                for h in range(4):
                    sl = slice(h * 2048, (h + 1) * 2048)
                    pf = pfp.tile([QB, 2048], F32, tag="pf")
                    nc.vector.tensor_scalar_mul(pf[:], e_s[:, sl], recip[:])
                    nc.sync.dma_start(p_d[r0 : r0 + QB, sl], pf[:])

            # PE stream per qb: mm1 pairs interleaved with the PREVIOUS
            # block's transposes (fills the ACT-paced exp window), then the
            # previous block's mm2 + outputs.
            prev = None
            for qb in range(NQB):
                adjf_s = adjp.tile([QB, N], BF16, tag="adjf")
                nc.sync.dma_start(adjf_s[:], adjf_d[qb * QB : (qb + 1) * QB, :])
                e_s = ep.tile([QB, N], BF16, tag="es")
                et_prev = None
                if prev is not None:
                    et_prev = etp.tile([128, NKC, KC], BF16, tag="ets")
                for kp in range(NKG // 2):
                    emit_mm1_pair(qb, e_s, adjf_s, kp)
                    if prev is not None:
                        emit_transp_group(prev[1], et_prev, kp)
                if prev is not None:
                    emit_mm2_outputs(prev[0], prev[1], et_prev)
                prev = (qb, e_s)

            # epilogue: last block's transposes + mm2
            et_last = etp.tile([128, NKC, KC], BF16, tag="ets")
            for g in range(NKC // 8):
                emit_transp_group(prev[1], et_last, g)
            emit_mm2_outputs(prev[0], prev[1], et_last)

    nc.compile()
    return nc


_NC_CACHE = None


def _get_nc():
    global _NC_CACHE
    if _NC_CACHE is None:
        _NC_CACHE = build_nc()
    return _NC_CACHE


def kernel(x, adj, Vw, Vb, _want_time=False):
    x = np.asarray(x, dtype=np.float32)
    adj_np = np.asarray(adj)
    Vw = np.asarray(Vw, dtype=np.float32)
    Vb = np.asarray(Vb, dtype=np.float32)

    # host-side prep (sharding + tiny value projection)
    value = x @ Vw.T + Vb                                    # [N, DIN] f32
    vaug = np.concatenate([value, np.ones((N, 1), np.float32)], axis=1)
    vaug_bf = np.ascontiguousarray(
        vaug.astype(ml_dtypes.bfloat16).reshape(NKC, KC, DAUG).transpose(1, 0, 2)
    )
    xt_bf = np.ascontiguousarray(
        x.T.astype(ml_dtypes.bfloat16).reshape(2, 128, N).transpose(1, 0, 2)
    )
    adj_bf = adj_np.astype(ml_dtypes.bfloat16)               # [N, N]

    in_maps = []
    for i in range(M):
        rows = slice(i * Q, (i + 1) * Q)
        xq = x[rows].T.astype(ml_dtypes.bfloat16)
        in_maps.append(
            {
                "xt": xt_bf,
                "xqt": np.ascontiguousarray(xq.reshape(2, 128, Q).transpose(1, 0, 2)),
                "vaug": vaug_bf,
                "adjf": np.ascontiguousarray(adj_bf[rows]),
            }
        )

    nc = _get_nc()
    res = run_bass_kernel_spmd(nc, in_maps, list(range(M)), trace=_want_time)

    p_attn = np.empty((N, N), np.float32)
    vs = np.empty((N, DIN), np.float32)
    for i in range(M):
        rows = slice(i * Q, (i + 1) * Q)
        p_attn[rows] = res.results[i]["p"]
        vs[rows] = res.results[i]["vs"]

    if _want_time:
        return (vs, p_attn), res.exec_time_ns
    return (vs, p_attn)
